# revision 1
# baseline (speedup 1.0000x reference)
"""ColumnParallelFusedMoeLinear grouped-GEMM kernel for 8 Trainium2 NeuronCores.

Strategy (expert/token parallel, not column parallel):
  Tokens are sorted by expert; m_sizes gives each expert's contiguous row
  range of x.  The host splits the full token range into 8 contiguous chunks,
  each served by one core, allocated proportionally to per-expert segment
  sizes so every chunk maps to exactly ONE expert (balanced m_sizes -> one
  expert per core; a skewed expert gets split across several cores along
  tokens).  Each core computes y_chunk = x_chunk @ weight[e].T as a dense
  matmul and the host scatters chunk rows back into the full output.

  Per-core HBM traffic = x_chunk (~4.5 MB) + one weight (8 MB) + y_chunk
  (~9 MB), which is the global minimum (x, weight, y each touched once
  across the chip).

  The matmuls run in float32r (fp32 data, PE rounds to 11-bit mantissa,
  full 1 column/cycle rate at free dim >= 256) with fp32 PSUM accumulation.

  Host pre-transposes x-chunks and weights so the contraction dim (D_IN)
  is the partition dim for both operands and every DMA is wide/contiguous.
"""

import math

import numpy as np

_N_CORES = 8
_P = 128
_NFREE = 512  # matmul moving free dim / PSUM bank width in fp32

# (M_pad, D_IN, D_OUT) -> (nc, in_names) compiled-program cache so repeated
# kernel() calls in one process reuse the traced module (and hence the
# process-level NEFF compile cache).
_program_cache = {}


def _build_program(m_pad, d_in, d_out, out_engine="gpsimd", w_merge_tail=False,
                   out_pair=False):
    import concourse.mybir as mybir
    import concourse.tile as tile
    from concourse import bacc

    kc_n = d_in // _P          # contraction chunks of 128
    mt_n = m_pad // _P         # token tiles
    nt_n = d_out // _NFREE     # output-column tiles

    nc = bacc.Bacc("TRN2", target_bir_lowering=False, debug=False)
    xT = nc.dram_tensor("xT", [d_in, m_pad], mybir.dt.float32r, kind="ExternalInput")
    wT = nc.dram_tensor("wT", [d_in, d_out], mybir.dt.float32r, kind="ExternalInput")
    y = nc.dram_tensor("y", [m_pad, d_out], mybir.dt.float32, kind="ExternalOutput")

    xT3 = xT.rearrange("(kc p) m -> kc p m", p=_P)
    wT3 = wT.rearrange("(kc p) o -> kc p o", p=_P)
    y3 = y.rearrange("(mt p) o -> mt p o", p=_P)

    # x columns are DMA'd in two groups per k-chunk; the compute loop runs the
    # head m-tiles through ALL weight columns first, then the tail m-tiles.
    # This spreads both the x and w input streams across the whole timeline
    # (peak early-bandwidth demand is what stalls the PE otherwise).
    XG_HEAD = min((mt_n + 1) // 2 + 1, mt_n)

    with tile.TileContext(nc) as tc:
        with (
            tc.tile_pool(name="xw", bufs=1) as xwpool,
            tc.tile_pool(name="out", bufs=8) as outpool,
            tc.tile_pool(name="psum", bufs=8, space="PSUM") as psumpool,
        ):
            wt = {}
            xh = [[None] * kc_n, [None] * kc_n]  # head / tail x tiles per kc

            def load_w(nts, kc):
                """One DMA covering weight columns nts (a contiguous list)."""
                n0, n1 = nts[0], nts[-1] + 1
                t = xwpool.tile([_P, (n1 - n0) * _NFREE], mybir.dt.float32r,
                                tag=f"w{kc}_{n0}")
                bi = nc.sync.dma_start(t[:], wT3[kc, :, n0 * _NFREE:n1 * _NFREE])
                in_dma_insts.append(bi.ins)
                for nt in nts:
                    wt[(kc, nt)] = t[:, (nt - n0) * _NFREE:(nt - n0 + 1) * _NFREE]

            def load_xh(h, kc):
                c0 = 0 if h == 0 else XG_HEAD * _P
                c1 = XG_HEAD * _P if h == 0 else m_pad
                t = xwpool.tile([_P, c1 - c0], mybir.dt.float32r, tag=f"x{kc}_{h}")
                bi = nc.sync.dma_start(t[:], xT3[kc, :, c0:c1])
                in_dma_insts.append(bi.ins)
                xh[h][kc] = t

            in_dma_insts = []

            # DMA emission in PE need-order.  Transfers serialize at ~full
            # HBM bandwidth, so program order == arrival order.  The first
            # k-loop needs x(kc, m0..) + w(kc, n0) pairwise, so interleave
            # those; the x remainder and later weight columns follow in
            # consumption order (the last two columns merged into one DMA —
            # they arrive with plenty of slack).
            for kc in range(kc_n):
                load_xh(0, kc)
                load_w([0], kc)
            if nt_n > 1:
                for kc in range(kc_n):
                    load_w([1], kc)
            if nt_n > 2:
                if w_merge_tail:
                    for kc in range(kc_n):
                        load_w(list(range(2, nt_n)), kc)
                else:
                    for nt in range(2, nt_n):
                        for kc in range(kc_n):
                            load_w([nt], kc)
            if mt_n > XG_HEAD:
                for kc in range(kc_n):
                    load_xh(1, kc)

            out_dma = {
                "gpsimd": nc.gpsimd.dma_start,
                "scalar": nc.scalar.dma_start,
                "scalar_ordered": nc.scalar.dma_start,
                "sync": nc.sync.dma_start,
            }[out_engine]
            last_in_dma = in_dma_insts[-1]
            halves = [(0, XG_HEAD)]
            if XG_HEAD < mt_n:
                halves.append((XG_HEAD, mt_n))
            for h0, h1 in halves:
              for nt in range(nt_n):
                mt = h0
                while mt < h1:
                    # pair adjacent m-tiles into one store tile
                    npair = 2 if (out_pair and mt + 1 < h1) else 1
                    o = outpool.tile([_P, npair * _NFREE], mybir.dt.float32, tag="o")
                    for j in range(npair):
                        r_mt = mt + j
                        if r_mt < XG_HEAD:
                            lhs_tile, r = xh[0], r_mt
                        else:
                            lhs_tile, r = xh[1], r_mt - XG_HEAD
                        ps = psumpool.tile([_P, _NFREE], mybir.dt.float32, tag="ps")
                        for kc in range(kc_n):
                            nc.tensor.matmul(
                                ps[:],
                                lhs_tile[kc][:, r * _P:(r + 1) * _P],
                                wt[(kc, nt)],
                                start=(kc == 0),
                                stop=(kc == kc_n - 1),
                            )
                        nc.vector.tensor_copy(o[:, j * _NFREE:(j + 1) * _NFREE], ps[:])
                    dst = y3[mt:mt + npair, :, nt * _NFREE:(nt + 1) * _NFREE]
                    bi = out_dma(
                        dst.rearrange("t p n -> p t n"),
                        o[:].rearrange("p (t n) -> p t n", t=npair),
                    )
                    if out_engine == "scalar_ordered":
                        # schedule-order all stores after every input load so
                        # the HWDGE lane rotation never chains a load behind
                        # a store
                        tile.add_dep_helper(
                            bi.ins, last_in_dma, sync=False,
                            reason="stores after loads for clean DMA lanes",
                        )
                    mt += npair
    nc.compile()
    return nc


# Largest chunk one core handles per SPMD round; 8 k-chunks of x at this
# width (4 B elements) stay well inside the 192 KB/partition SBUF budget
# next to the resident weight tiles.
_MAX_CHUNK = 2560


def _plan_chunks(m_sizes, T):
    """Split [0, T) into single-expert chunks, balanced by length.

    Every chunk is <= _MAX_CHUNK rows.  Returns a list of (expert, row0,
    row1) padded with empty (0, 0, 0) chunks to a multiple of _N_CORES,
    or None if there are no rows at all.
    """
    off = np.cumsum(np.asarray(m_sizes, dtype=np.int64))
    starts = np.clip(np.concatenate([[0], off[:-1]]), 0, T)
    ends = np.clip(off, 0, T)
    segs = [(e, int(starts[e]), int(ends[e]))
            for e in range(len(m_sizes)) if ends[e] > starts[e]]
    if not segs:
        return None
    lens = np.array([s1 - s0 for _, s0, s1 in segs], dtype=np.float64)
    # mandatory splits so no chunk exceeds _MAX_CHUNK, then distribute any
    # spare cores (up to the next multiple of _N_CORES) to the biggest shares
    n_chunks = np.ceil(lens / _MAX_CHUNK).astype(np.int64)
    total = int(n_chunks.sum())
    spare = (-total) % _N_CORES if total > _N_CORES else _N_CORES - total
    for _ in range(spare):
        i = int(np.argmax(lens / n_chunks))
        n_chunks[i] += 1
    chunks = []
    for (e, s0, s1), k in zip(segs, n_chunks):
        L = s1 - s0
        bounds = [s0 + (L * i) // k for i in range(int(k) + 1)]
        for i in range(int(k)):
            if bounds[i + 1] > bounds[i]:
                chunks.append((e, bounds[i], bounds[i + 1]))
    while len(chunks) % _N_CORES:
        chunks.append((0, 0, 0))
    return chunks


def kernel(x, weight, m_sizes):
    from concourse.bass_utils import run_bass_kernel_spmd

    x = np.ascontiguousarray(np.asarray(x), dtype=np.float32)
    weight = np.ascontiguousarray(np.asarray(weight), dtype=np.float32)
    m_arr = np.asarray(m_sizes)

    T, d_in = x.shape
    E, d_out, _ = weight.shape

    y = np.zeros((T, d_out), dtype=np.float32)
    chunks = _plan_chunks(m_arr, T)
    if chunks is None:
        return y

    max_len = max(r1 - r0 for _, r0, r1 in chunks)
    m_pad = max(_P, int(math.ceil(max_len / _P)) * _P)

    import os
    out_engine = os.environ.get("MOE_OUT_ENGINE", "scalar_ordered")
    w_merge_tail = os.environ.get("MOE_W_MERGE", "1") == "1"
    out_pair = os.environ.get("MOE_OUT_PAIR", "1") == "1"
    key = (m_pad, d_in, d_out, out_engine, w_merge_tail, out_pair)
    if key not in _program_cache:
        _program_cache[key] = _build_program(
            m_pad, d_in, d_out, out_engine, w_merge_tail, out_pair
        )
    nc = _program_cache[key]

    # weight[e].T, C-contiguous, built once per expert actually used
    wT_cache = {}
    for round0 in range(0, len(chunks), _N_CORES):
        batch = chunks[round0:round0 + _N_CORES]
        in_maps = []
        for e, r0, r1 in batch:
            xT = np.zeros((d_in, m_pad), dtype=np.float32)
            if r1 > r0:
                xT[:, : r1 - r0] = x[r0:r1].T
            if e not in wT_cache:
                wT_cache[e] = np.ascontiguousarray(weight[e].T)
            in_maps.append({"xT": xT, "wT": wT_cache[e]})

        res = run_bass_kernel_spmd(nc, in_maps, core_ids=list(range(_N_CORES)))

        for (e, r0, r1), out in zip(batch, res.results):
            if r1 > r0:
                y[r0:r1] = out["y"][: r1 - r0]
    return y

